# revision 1
# baseline (speedup 1.0000x reference)
"""MoE downsample kernel for 8 TRN2 NeuronCores.

Strategy: data-parallel over batch (2 samples per core). Each strided dilated
conv is decomposed into k*k "tap" matmuls (lhsT = [Cin=64, Cout=64] weight
slice, rhs = strided view of the zero-padded input image) accumulated in PSUM
over 512-pixel output chunks. Samples map to PE-array row halves (partitions
0-63 / 64-127) and two balanced expert queues map to PE col halves, so four
64x64 tile_position matmuls run concurrently (full 128x128 array).
BN + conv-bias + GELU are fused into the ScalarE PSUM eviction.
Gating (tiny: 16x64x4 matvec + softmax + top2) and final top-2 assembly run
on host.
"""

import numpy as np
import ml_dtypes

KS = [3, 5, 7, 9]
DS = [1, 2, 3, 4]
BN_EPS = 1e-5
B, CIN, H, W = 16, 64, 256, 256
CE = 64
PAD = 16          # left/top pad (max |offset|); right/bottom needs 15
HP = WP = PAD + 256 + 15   # 287
HO = WO = 128
NCORES = 8
SPC = 2           # samples per core
NTAPS = sum(k * k for k in KS)  # 164
CHUNK_ROWS = 4    # output rows per 512-px chunk
NCHUNKS = HO // CHUNK_ROWS      # 32

# tap slot base per expert
_SLOT_BASE = np.cumsum([0] + [k * k for k in KS]).tolist()

# queue split: col0 = experts [0,1,2] (83 taps), col1 = [3] (81 taps)
COL_EXPERTS = [[0, 1, 2], [3]]

_COMPILED = None


def _tap_offsets(e):
    """Yield (slot, row_off, col_off) in padded coords for expert e, tap (u,v)."""
    k, d = KS[e], DS[e]
    pad = d * (k - 1) // 2
    for u in range(k):
        for v in range(k):
            slot = _SLOT_BASE[e] + u * k + v
            yield slot, d * u - pad + PAD, d * v - pad + PAD


def _build_program():
    import concourse.bass as bass  # noqa: F401
    import concourse.mybir as mybir
    import concourse.tile as tile
    from concourse import bacc
    from contextlib import ExitStack

    dt = mybir.dt
    nc = bacc.Bacc("TRN2", target_bir_lowering=False, debug=False,
                   num_devices=NCORES)
    xpad = nc.dram_tensor("xpad", [SPC, CIN, HP, WP], dt.bfloat16,
                          kind="ExternalInput")
    wt = nc.dram_tensor("wt", [CIN, NTAPS, CE], dt.bfloat16,
                        kind="ExternalInput")
    bnp = nc.dram_tensor("bnp", [CE, 4, 2], dt.float32, kind="ExternalInput")
    out = nc.dram_tensor("out", [SPC, 4, CE, HO, WO], dt.float32,
                         kind="ExternalOutput")

    with tile.TileContext(nc) as tc:
        with ExitStack() as ctx:
            consts = ctx.enter_context(tc.tile_pool(name="consts", bufs=1))
            stage_pool = ctx.enter_context(tc.tile_pool(name="st", bufs=8))

            # ---- load constants / inputs into SBUF ----
            # first x stripe (rows needed by chunk 0) goes out first so the
            # PE can start as early as possible; weights ride alongside.
            wtile = consts.tile([128, NTAPS, CE], dt.bfloat16)
            bntile = consts.tile([128, 4, 2], dt.float32)
            xtile = consts.tile([128, HP, WP], dt.bfloat16)
            bounds = [0, 44] + [44 + ((HP - 44) * p) // 14
                                for p in range(1, 15)]
            for s in range(SPC):
                nc.gpsimd.dma_start(
                    out=xtile[s * 64:(s + 1) * 64, 0:44, :],
                    in_=xpad[s, :, 0:44, :])
            for half in range(2):
                p0 = half * 64
                nc.gpsimd.dma_start(out=wtile[p0:p0 + 64, :, :], in_=wt.ap())
                nc.gpsimd.dma_start(out=bntile[p0:p0 + 64, :, :], in_=bnp.ap())
            for piece in range(1, len(bounds) - 1):
                for s in range(SPC):
                    r0, r1 = bounds[piece], bounds[piece + 1]
                    nc.gpsimd.dma_start(
                        out=xtile[s * 64:(s + 1) * 64, r0:r1, :],
                        in_=xpad[s, :, r0:r1, :])

            psum_pool = ctx.enter_context(
                tc.tile_pool(name="ps", bufs=8, space="PSUM"))

            # ---- main loop: 32 chunks x (2 samples x 2 col-queues) ----
            def queue_events(s, col, r):
                p0 = s * 64           # rhs/lhsT partition base (PE rows)
                q0 = col * 64         # psum/out partition base (PE cols)
                i0 = r * CHUNK_ROWS   # first output row
                for e in COL_EXPERTS[col]:
                    ps = psum_pool.tile([128, 512], dt.float32)
                    taps = list(_tap_offsets(e))
                    for t, (slot, ro, co) in enumerate(taps):
                        r_lo = 2 * i0 + ro
                        rhs = xtile[p0:p0 + 64,
                                    r_lo:r_lo + 2 * CHUNK_ROWS - 1:2,
                                    co:co + 2 * WO - 1:2]
                        lhsT = wtile[p0:p0 + 64, slot, :]
                        psv = ps[q0:q0 + 64, :]
                        first = t == 0
                        last = t == len(taps) - 1

                        def mm(rhs=rhs, lhsT=lhsT, psv=psv, first=first,
                               last=last, p0=p0, q0=q0):
                            nc.tensor.matmul(psv, lhsT, rhs, start=first,
                                             stop=last,
                                             tile_position=(p0, q0))
                        yield ("mm", mm)

                    def evict(ps=ps, s=s, e=e, i0=i0, q0=q0):
                        st = stage_pool.tile([128, CHUNK_ROWS, WO],
                                             dt.float32)
                        nc.scalar.activation(
                            st[q0:q0 + 64, :, :],
                            ps[q0:q0 + 64, :].rearrange(
                                "p (a b) -> p a b", a=CHUNK_ROWS),
                            mybir.ActivationFunctionType.Gelu,
                            scale=bntile[q0:q0 + 64, e, 0:1],
                            bias=bntile[q0:q0 + 64, e, 1:2])
                        nc.sync.dma_start(
                            out=out[s, e, :, i0:i0 + CHUNK_ROWS, :],
                            in_=st[q0:q0 + 64, :, :])
                    yield ("evict", evict)

            for r in range(NCHUNKS):
                queues = [queue_events(s, col, r)
                          for s in range(SPC) for col in range(2)]
                live = list(queues)
                while live:
                    nxt = []
                    for q in live:
                        ev = next(q, None)
                        if ev is None:
                            continue
                        ev[1]()
                        nxt.append(q)
                    live = nxt

    nc.compile()
    return nc


def _get_program():
    global _COMPILED
    if _COMPILED is None:
        _COMPILED = _build_program()
    return _COMPILED


def _host_gate(x, gate_w, gate_b):
    """Replicate reference gating in numpy (f64 pooling for robustness)."""
    pooled = x.astype(np.float64).mean(axis=(2, 3)).astype(np.float32)
    logits = pooled @ gate_w.T.astype(np.float32) + gate_b
    z = logits - logits.max(axis=1, keepdims=True)
    ez = np.exp(z.astype(np.float32))
    gates = ez / ez.sum(axis=1, keepdims=True)
    idx = np.argsort(-gates, axis=1, kind="stable")[:, :2]
    wsel = np.take_along_axis(gates, idx, axis=1)
    wsel = wsel / (wsel.sum(axis=1, keepdims=True) + 1e-8)
    return idx, wsel.astype(np.float32)


def _prep_inputs(x, ws, bs, bn_scale, bn_bias, bn_mean, bn_var):
    bf16 = ml_dtypes.bfloat16
    # padded bf16 images, per core
    xpad = np.zeros((B, CIN, HP, WP), dtype=bf16)
    xpad[:, :, PAD:PAD + H, PAD:PAD + W] = x.astype(bf16)

    # transposed weights, DMA-friendly layout [CIN, NTAPS, CE]
    wt = np.empty((CIN, NTAPS, CE), dtype=bf16)
    for e in range(4):
        k = KS[e]
        w = ws[e].astype(np.float32)  # [CE, CIN, k, k]
        # [CE, CIN, k, k] -> [CIN, k*k, CE]
        wt[:, _SLOT_BASE[e]:_SLOT_BASE[e] + k * k, :] = (
            w.transpose(1, 2, 3, 0).reshape(CIN, k * k, CE).astype(bf16))

    # folded BN: z = conv*scale + shift ; scale = bn_scale*rsqrt(var+eps),
    # shift = conv_bias*scale + bn_bias - mean*scale
    inv = (bn_scale / np.sqrt(bn_var + BN_EPS)).astype(np.float32)
    shift = (np.stack(bs) * inv + bn_bias - bn_mean * inv).astype(np.float32)
    bnp = np.stack([inv, shift], axis=1)  # [4, 2, CE]
    bnp = np.ascontiguousarray(bnp.transpose(2, 0, 1))  # [CE, 4, 2]
    return xpad, wt, bnp


def run(inputs, trace=False):
    from concourse import bass_utils

    x = np.asarray(inputs["x"], dtype=np.float32)
    ws = [np.asarray(inputs[f"w{i}"], dtype=np.float32) for i in range(4)]
    bs = [np.asarray(inputs[f"b{i}"], dtype=np.float32) for i in range(4)]
    bn_scale = np.asarray(inputs["bn_scale"], dtype=np.float32)
    bn_bias = np.asarray(inputs["bn_bias"], dtype=np.float32)
    bn_mean = np.asarray(inputs["bn_mean"], dtype=np.float32)
    bn_var = np.asarray(inputs["bn_var"], dtype=np.float32)
    gate_w = np.asarray(inputs["gate_w"], dtype=np.float32)
    gate_b = np.asarray(inputs["gate_b"], dtype=np.float32)

    nc = _get_program()
    xpad, wt, bnp = _prep_inputs(x, ws, bs, bn_scale, bn_bias, bn_mean,
                                 bn_var)
    in_maps = []
    for c in range(NCORES):
        in_maps.append({
            "xpad": xpad[c * SPC:(c + 1) * SPC],
            "wt": wt,
            "bnp": bnp,
        })
    res = bass_utils.run_bass_kernel_spmd(
        nc, in_maps, core_ids=list(range(NCORES)), trace=trace)

    # assemble: E[b, e] for all experts, then host top-2 select/scale/concat
    E = np.concatenate([res.results[c]["out"] for c in range(NCORES)],
                       axis=0)  # [B, 4, CE, HO, WO]
    idx, wsel = _host_gate(x, gate_w, gate_b)
    outf = np.empty((B, 2 * CE, HO, WO), dtype=np.float32)
    for b in range(B):
        outf[b, :CE] = E[b, idx[b, 0]] * wsel[b, 0]
        outf[b, CE:] = E[b, idx[b, 1]] * wsel[b, 1]
    return outf, res


def kernel(**inputs):
    outf, _ = run(inputs, trace=False)
    return outf



# revision 6
# speedup vs baseline: 1.4778x; 1.4778x over previous
"""MoE downsample kernel for 8 TRN2 NeuronCores — top-2-only compute.

The reference output keeps only each sample's top-2 (of 4) experts, so the
kernel computes just those: the gate runs on host (tiny matvec + softmax),
then only selected (sample, expert) conv jobs are dispatched to the device,
roughly halving PE work vs dense all-expert compute.

SPMD constraint: one instruction stream for all 8 cores. The per-queue slot
schedule is derived ONLY from the global expert histogram (n0..n3): every
queue processes floor(n_e/2) two-chunk slots plus (n_e odd) one single-chunk
slot per expert, so all 32 queues (8 cores x 4 PE-quadrant queues) execute
identical tap/matmul sequences. Which physical (sample, chunk) each slot
handles is encoded purely in DATA: the host stages each slot's input rows
into a canonical per-core stripe buffer.

Each strided dilated conv decomposes into k*k tap matmuls (lhsT=[64,64]
weight slice vs strided stripe view) accumulated in PSUM per 512-px chunk.
Four 64x64 tile_position quadrant matmuls run concurrently. Taps are outer,
chunks inner (weight-stationary within a slot) to amortize LDWEIGHTS.
BN + conv-bias + GELU fuse into the ScalarE PSUM eviction; the per-sample
gate weights and top-2 concat run on host.
"""

import numpy as np
import ml_dtypes

KS = [3, 5, 7, 9]
DS = [1, 2, 3, 4]
BN_EPS = 1e-5
B, CIN, H, W = 16, 64, 256, 256
CE = 64
PAD = 16                    # left/top pad (max pad_e); right/bottom needs 15
HP = WP = PAD + 256 + 15    # 287
HO = WO = 128
NCORES = 8
NTAPS = sum(k * k for k in KS)  # 164
NCHUNKS = 32                # 4-output-row chunks per (sample, expert) job

_SLOT_BASE = np.cumsum([0] + [k * k for k in KS]).tolist()
_PADE = [DS[e] * (KS[e] - 1) // 2 for e in range(4)]

_COMPILED = {}


def _rows(e, L):
    """Stripe rows needed for L consecutive chunks of expert e."""
    return 8 * L + DS[e] * (KS[e] - 1) - 1


def _qsched(hist):
    """Per-queue slot list [(e, L)] — identical for all 32 queues.

    Ordered heavy/light alternating to spread ScalarE evictions."""
    slots = []
    for e in range(4):
        slots += [(e, 2)] * (hist[e] // 2)
        slots += [(e, 1)] * (hist[e] % 2)
    slots.sort(key=lambda s: -(KS[s[0]] ** 2 * s[1]))
    heavy, light = slots[: len(slots) // 2], slots[len(slots) // 2:]
    order = []
    for i in range(max(len(heavy), len(light))):
        if i < len(heavy):
            order.append(heavy[i])
        if i < len(light):
            order.append(light[i])
    return order


def _tap_offsets(e):
    """Yield (wslot, u, v, co) — co is the padded-coords column offset."""
    k, d = KS[e], DS[e]
    pad = _PADE[e]
    for u in range(k):
        for v in range(k):
            yield _SLOT_BASE[e] + u * k + v, u, v, d * v - pad + PAD


def _build_program(hist):
    import concourse.bass as bass  # noqa: F401
    import concourse.mybir as mybir
    import concourse.tile as tile
    from concourse import bacc
    from contextlib import ExitStack

    sched = _qsched(hist)
    R = sum(_rows(e, L) for e, L in sched)
    nchunks_core = 4 * sum(L for _, L in sched)

    dt = mybir.dt
    nc = bacc.Bacc("TRN2", target_bir_lowering=False, debug=False,
                   num_devices=NCORES)
    xs = nc.dram_tensor("xs", [4, CIN, R, WP], dt.bfloat16,
                        kind="ExternalInput")
    wt = nc.dram_tensor("wt", [CIN, NTAPS, CE], dt.bfloat16,
                        kind="ExternalInput")
    bnp = nc.dram_tensor("bnp", [CE, 4, 2], dt.float32, kind="ExternalInput")
    ys = nc.dram_tensor("ys", [nchunks_core, CE, 4, WO], dt.float32,
                        kind="ExternalOutput")

    with tile.TileContext(nc) as tc:
        with ExitStack() as ctx:
            consts = ctx.enter_context(tc.tile_pool(name="consts", bufs=1))
            stripe_pool = ctx.enter_context(tc.tile_pool(name="xp", bufs=6))
            st_pool = ctx.enter_context(tc.tile_pool(name="st", bufs=8))
            psum_pool = ctx.enter_context(
                tc.tile_pool(name="ps", bufs=8, space="PSUM"))

            wtile = consts.tile([128, NTAPS, CE], dt.bfloat16)
            bntile = consts.tile([128, 4, 2], dt.float32)
            for half in range(2):
                p0 = half * 64
                nc.gpsimd.dma_start(out=wtile[p0:p0 + 64, :, :], in_=wt.ap())
                nc.gpsimd.dma_start(out=bntile[p0:p0 + 64, :, :],
                                    in_=bnp.ap())

            roffs = np.cumsum([0] + [_rows(e, L) for e, L in sched]).tolist()

            def load_stripes(j):
                e, L = sched[j]
                rows = _rows(e, L)
                r0 = roffs[j]
                tiles = []
                for pair in range(2):   # A: queues 0/2, B: queues 1/3
                    t = stripe_pool.tile([128, rows, WP], dt.bfloat16)
                    for h in range(2):
                        q = 2 * h + pair
                        nc.gpsimd.dma_start(
                            out=t[h * 64:h * 64 + 64, :, :],
                            in_=xs[q, :, r0:r0 + rows, :])
                    tiles.append(t)
                return tiles

            stripes = {0: load_stripes(0)}
            if len(sched) > 1:
                stripes[1] = load_stripes(1)

            g = 0
            for j, (e, L) in enumerate(sched):
                if j + 2 < len(sched):
                    stripes[j + 2] = load_stripes(j + 2)
                tA, tB = stripes.pop(j)
                d = DS[e]
                taps = list(_tap_offsets(e))
                # psum tiles: [pair][i] — pair A: q0(p0)+q1(p1); B: q2+q3
                # psum tile pss[h][i]: parts 0-63 <- queue (h,p=0),
                # parts 64-127 <- queue (h,p=1). stripe tiles[p]: parts
                # 0-63 <- queue (h=0,p), parts 64-127 <- queue (h=1,p).
                pss = [[psum_pool.tile([128, 512], dt.float32, name="ps")
                        for i in range(L)] for h in range(2)]
                for t, (wslot, u, v, co) in enumerate(taps):
                    first = t == 0
                    last = t == len(taps) - 1
                    for i in range(L):
                        rlo = 8 * i + d * u
                        for q in range(4):
                            h, p = q // 2, q % 2
                            stile = tA if p == 0 else tB
                            nc.tensor.matmul(
                                pss[h][i][p * 64:p * 64 + 64, :],
                                wtile[h * 64:h * 64 + 64, wslot, :],
                                stile[h * 64:h * 64 + 64, rlo:rlo + 7:2,
                                      co:co + 2 * WO - 1:2],
                                start=first, stop=last,
                                tile_position=(h * 64, p * 64))
                for i in range(L):
                    for h in range(2):
                        ps = pss[h][i]
                        st = st_pool.tile([128, 4, WO], dt.float32)
                        for p in range(2):
                            p0 = p * 64
                            nc.scalar.activation(
                                st[p0:p0 + 64, :, :],
                                ps[p0:p0 + 64, :].rearrange(
                                    "p (a b) -> p a b", a=4),
                                mybir.ActivationFunctionType.Gelu,
                                scale=bntile[p0:p0 + 64, e, 0:1],
                                bias=bntile[p0:p0 + 64, e, 1:2])
                            nc.sync.dma_start(
                                out=ys[g, :, :, :],
                                in_=st[p0:p0 + 64, :, :])
                            g += 1

    nc.compile()
    return nc, sched, roffs, nchunks_core


def _get_program(hist):
    key = tuple(hist)
    if key not in _COMPILED:
        _COMPILED[key] = _build_program(hist)
    return _COMPILED[key]


def _host_gate(x, gate_w, gate_b):
    pooled = x.astype(np.float64).mean(axis=(2, 3)).astype(np.float32)
    logits = pooled @ gate_w.T.astype(np.float32) + gate_b
    z = logits - logits.max(axis=1, keepdims=True)
    ez = np.exp(z.astype(np.float32))
    gates = ez / ez.sum(axis=1, keepdims=True)
    idx = np.argsort(-gates, axis=1, kind="stable")[:, :2]
    wsel = np.take_along_axis(gates, idx, axis=1)
    wsel = wsel / (wsel.sum(axis=1, keepdims=True) + 1e-8)
    return idx, wsel.astype(np.float32)


def _assign_jobs(idx, sched):
    """Deal (sample, rank, chunk) work to the 32 queues.

    Returns per-queue slot job lists: jobs[32][len(sched)] = (b, r, c0)."""
    pools = {e: [] for e in range(4)}   # per expert: list of (b, r)
    for b in range(B):
        for r in range(2):
            pools[idx[b, r]].append((b, r))
    # build pair/single supplies per expert
    pairs = {e: [(b, r, c0) for (b, r) in pools[e]
                 for c0 in range(0, NCHUNKS, 2)] for e in range(4)}
    singles = {e: [] for e in range(4)}
    for e in range(4):
        if len(pools[e]) % 2:
            take = pairs[e][-16:]
            pairs[e] = pairs[e][:-16]
            for (b, r, c0) in take:
                singles[e] += [(b, r, c0), (b, r, c0 + 1)]
    jobs = [[None] * len(sched) for _ in range(32)]
    pi = {e: 0 for e in range(4)}
    si = {e: 0 for e in range(4)}
    for qq in range(32):
        for j, (e, L) in enumerate(sched):
            if L == 2:
                jobs[qq][j] = pairs[e][pi[e]]
                pi[e] += 1
            else:
                jobs[qq][j] = singles[e][si[e]]
                si[e] += 1
    for e in range(4):
        assert pi[e] == len(pairs[e]) and si[e] == len(singles[e])
    return jobs


def _prep_weights(ws, bs, bn_scale, bn_bias, bn_mean, bn_var):
    bf16 = ml_dtypes.bfloat16
    wt = np.empty((CIN, NTAPS, CE), dtype=bf16)
    for e in range(4):
        k = KS[e]
        w = ws[e].astype(np.float32)  # [CE, CIN, k, k]
        wt[:, _SLOT_BASE[e]:_SLOT_BASE[e] + k * k, :] = (
            w.transpose(1, 2, 3, 0).reshape(CIN, k * k, CE).astype(bf16))
    inv = (bn_scale / np.sqrt(bn_var + BN_EPS)).astype(np.float32)
    shift = (np.stack(bs) * inv + bn_bias - bn_mean * inv).astype(np.float32)
    bnp = np.stack([inv, shift], axis=1)            # [4, 2, CE]
    bnp = np.ascontiguousarray(bnp.transpose(2, 0, 1))  # [CE, 4, 2]
    return wt, bnp


def run(inputs, trace=False):
    from concourse import bass_utils

    x = np.asarray(inputs["x"], dtype=np.float32)
    ws = [np.asarray(inputs[f"w{i}"], dtype=np.float32) for i in range(4)]
    bs = [np.asarray(inputs[f"b{i}"], dtype=np.float32) for i in range(4)]
    bn_scale = np.asarray(inputs["bn_scale"], dtype=np.float32)
    bn_bias = np.asarray(inputs["bn_bias"], dtype=np.float32)
    bn_mean = np.asarray(inputs["bn_mean"], dtype=np.float32)
    bn_var = np.asarray(inputs["bn_var"], dtype=np.float32)
    gate_w = np.asarray(inputs["gate_w"], dtype=np.float32)
    gate_b = np.asarray(inputs["gate_b"], dtype=np.float32)

    idx, wsel = _host_gate(x, gate_w, gate_b)
    hist = [int(np.sum(idx == e)) for e in range(4)]
    nc, sched, roffs, nchunks_core = _get_program(hist)
    jobs = _assign_jobs(idx, sched)
    wt, bnp = _prep_weights(ws, bs, bn_scale, bn_bias, bn_mean, bn_var)

    bf16 = ml_dtypes.bfloat16
    xpad = np.zeros((B, CIN, HP, WP), dtype=bf16)
    xpad[:, :, PAD:PAD + H, PAD:PAD + W] = x.astype(bf16)

    R = roffs[-1]
    in_maps = []
    for core in range(NCORES):
        xsb = np.empty((4, CIN, R, WP), dtype=bf16)
        for q in range(4):
            qq = core * 4 + q
            for j, (e, L) in enumerate(sched):
                b, r, c0 = jobs[qq][j]
                rows = _rows(e, L)
                row0 = 8 * c0 + PAD - _PADE[e]
                xsb[q, :, roffs[j]:roffs[j] + rows, :] = (
                    xpad[b, :, row0:row0 + rows, :])
        in_maps.append({"xs": xsb, "wt": wt, "bnp": bnp})

    res = bass_utils.run_bass_kernel_spmd(
        nc, in_maps, core_ids=list(range(NCORES)), trace=trace)

    # assemble output: replay schedule to map ys chunks back
    outf = np.zeros((B, 2 * CE, HO, WO), dtype=np.float32)
    for core in range(NCORES):
        ysb = res.results[core]["ys"]  # [nchunks_core, CE, 4, WO]
        g = 0
        for j, (e, L) in enumerate(sched):
            for i in range(L):
                for h in range(2):
                    for p in range(2):
                        q = 2 * h + p
                        b, r, c0 = jobs[core * 4 + q][j]
                        c = c0 + i
                        outf[b, 64 * r:64 * r + 64, 4 * c:4 * c + 4, :] = \
                            ysb[g]
                        g += 1
    for b in range(B):
        outf[b, :CE] *= wsel[b, 0]
        outf[b, CE:] *= wsel[b, 1]
    return outf, res


def kernel(**inputs):
    outf, _ = run(inputs, trace=False)
    return outf
